# revision 89
# baseline (speedup 1.0000x reference)
"""Trainium2 Bass kernel for nn_KernelFilter_S (dynamic per-sample filter CNN).

Data-parallel over batch B=8 across 8 NeuronCores (one sample per core).

Per-core math (x = content[b], s = style[b]):
  c1 = conv3x3(x, ds_w) + ds_b                       [32,64,64]
  pooled_F = mean_HW(conv3x3(s, cwF)) + cbF          [32]    (F = 1,2)
  filtF = (pooled_F @ fwF.T + fbF).reshape(32,32,3,3)
  c2 = leaky(conv3x3_dyn(c1, filt1), 0.2)
  c3 = conv3x3_dyn(c2, filt2)
  out = x + conv3x3(c3, up_w) + up_b                 [512,64,64]

Structure (v2):
  * mean-pool-of-conv -> 9 rectangle sums R[i,t] per style channel
    (reductions only, no style convs).
  * all convs as PSUM-accumulated matmuls over zero-padded 66x66 images,
    8 row-tiles of 8 rows -> N=512 (full PSUM bank) via 2D access patterns.
  * ds conv (512->32): PE column tiling - 4 concurrent matmuls in 32-col
    PE sub-arrays (tile_position), partials summed during drain.
  * dyn/up convs (K=32 per tap): kx-shift K-packing - inputs materialized
    as X3[kx*32+i, p] = img[i, p+kx-1], so one matmul contracts (kx,i)=96
    rows and ky becomes a free AP row offset: 3 matmuls instead of 9.
  * filter predictor: pooled via 36 accumulated matmuls (both filters
    merged, M=64); FC as 18 N=512 matmuls with bias folded in via
    ones-rows; filters reshaped through a DRAM round-trip.
  * content DMAed straight into the padded layout (border-only memsets),
    bf16 in/out; fp32 accumulation everywhere.
"""

import os
import sys
import numpy as np

sys.path.insert(0, "/opt/trn_rl_repo")

import concourse.bass as bass
import concourse.bacc as bacc
import concourse.mybir as mybir
import concourse.tile as tile
from concourse.bass_utils import run_bass_kernel_spmd

F32 = mybir.dt.float32
BF16 = mybir.dt.bfloat16
FP8 = mybir.dt.float8e4
NP_BF16 = np.dtype(mybir.dt.np(BF16))
NP_FP8 = np.dtype(mybir.dt.np(FP8))

H = W = 64
PW = W + 2              # padded row width = 66
NPIX = H * W            # 4096
NPAD = (H + 2) * PW     # 66*66 = 4356
CIN = 512
INNER = 32
NCH = CIN // 128        # 4
NRT = 8                 # row tiles
TR = 8                  # rows per tile
NT = TR * W             # 512 = psum bank
ADD = mybir.AluOpType.add


def _yx(ap):
    return ap.rearrange("p (y x) -> p y x", x=PW)


def _build_program():
    nc = bacc.Bacc(None, target_bir_lowering=False)

    content_h = nc.dram_tensor("content", [CIN, NPAD], FP8, kind="ExternalInput")
    contf_h = nc.dram_tensor("contf", [CIN, NPIX], BF16, kind="ExternalInput")
    style_h = nc.dram_tensor("style", [CIN, NPIX], FP8, kind="ExternalInput")
    w_ds_h = nc.dram_tensor("w_ds", [128, 36 * INNER], FP8, kind="ExternalInput")
    cw_h = nc.dram_tensor("cw", [128, 36 * 64], BF16, kind="ExternalInput")
    w_up_h = nc.dram_tensor("w_up", [96, 3 * NCH * 128], BF16, kind="ExternalInput")
    fw_h = nc.dram_tensor("fw", [66, 9216], FP8, kind="ExternalInput")
    dsb_h = nc.dram_tensor("ds_b", [INNER], F32, kind="ExternalInput")
    upb_h = nc.dram_tensor("up_b", [128, NCH], F32, kind="ExternalInput")
    cb_h = nc.dram_tensor("cb", [64], F32, kind="ExternalInput")
    lfi_h = nc.dram_tensor("lfinit", [66, 2], FP8, kind="ExternalInput")
    id_h = nc.dram_tensor("ident", [128, 128], BF16, kind="ExternalInput")
    out_h = nc.dram_tensor("out", [CIN, NPIX], BF16, kind="ExternalOutput")
    fsc_h = nc.dram_tensor("fscratch", [2, 9216], BF16, kind="Internal")

    with tile.TileContext(nc) as tc:
        with (
            tc.tile_pool(name="const", bufs=1) as const,
            tc.tile_pool(name="img", bufs=1) as img,
            tc.tile_pool(name="sx", bufs=4) as sx,
            tc.tile_pool(name="work", bufs=2) as work,
            tc.tile_pool(name="drain", bufs=4) as drain,
            tc.tile_pool(name="cps", bufs=3, space=bass.MemorySpace.PSUM) as cps,
            tc.tile_pool(name="dps", bufs=2, space=bass.MemorySpace.PSUM) as dps,
            tc.tile_pool(name="pps", bufs=1, space=bass.MemorySpace.PSUM) as pps_pool,
            tc.tile_pool(name="fps", bufs=2, space=bass.MemorySpace.PSUM) as fps_pool,
        ):
            # ---- w_ds + style first (sync queue): longest dep chains ----
            w_ds_sb = const.tile([128, 36 * INNER], FP8, tag="wds")
            nc.sync.dma_start(out=w_ds_sb[:], in_=w_ds_h[:])
            styl = []
            for c in range(NCH):
                st = sx.tile([128, NPIX], FP8, tag="sx")
                nc.sync.dma_start(
                    out=st[:], in_=style_h[:].rearrange("(c p) q -> c p q", p=128)[c]
                )
                styl.append(st)
            cw_sb = const.tile([128, 36 * 64], BF16, tag="cw")
            nc.sync.dma_start(out=cw_sb[:], in_=cw_h[:])
            dsb_sb = const.tile([INNER, 1], F32, tag="dsb")
            nc.sync.dma_start(out=dsb_sb[:], in_=dsb_h[:].rearrange("(o u) -> o u", u=1))
            upb_sb = const.tile([128, NCH], F32, tag="upb")
            nc.sync.dma_start(out=upb_sb[:], in_=upb_h[:])
            cb_sb = const.tile([64, 1], F32, tag="cb")
            nc.sync.dma_start(out=cb_sb[:], in_=cb_h[:].rearrange("(o u) -> o u", u=1))
            id_sb = const.tile([128, 128], BF16, tag="ident")
            nc.sync.dma_start(out=id_sb[:], in_=id_h[:])

            # ---- PE warm-up (keeps HAM busy before content arrives) ------
            # depends only on w_ds, the first DMA the ds conv needs anyway
            warm_ps = cps.tile([128, NT], F32, tag="cps")
            for i in range(16):
                nc.tensor.matmul(
                    warm_ps[0:32], w_ds_sb[:, 0:32], w_ds_sb[:, 0:NT],
                    start=True, stop=True,
                )

            # ---- padded content images: border memset + direct DMA ------
            def border_memsets(t, eng1, eng2, wide):
                # top/bottom rows + left/right border cols of the 66x66 grid
                # wide=2 zeroes cols {0,65}; wide=4 zeroes cols {0,1,64,65}
                hw = wide // 2
                eng1.memset(t[:, 0:PW + hw], 0.0)
                off = PW - hw
                eng2.memset(
                    t[:, off:off + 65 * PW]
                    .rearrange("p (a b) -> p a b", b=PW)[:, :, 0:wide],
                    0.0,
                )
                eng1.memset(t[:, NPAD - PW - hw:NPAD], 0.0)

            # content (fp8, host-padded) for the conv path: contiguous DMAs
            cpad = []
            for c in range(NCH):
                cp = img.tile([128, NPAD], FP8, tag=f"cpad{c}")
                cpad.append(cp)
                nc.scalar.dma_start(
                    out=cp[:],
                    in_=content_h[:].rearrange("(c p) q -> c p q", p=128)[c],
                )

            fw_sb = const.tile([66, 9216], FP8, tag="fw")
            nc.sync.dma_start(out=fw_sb[:], in_=fw_h[:])
            w_up_sb = const.tile([96, 3 * NCH * 128], BF16, tag="wup")
            nc.sync.dma_start(out=w_up_sb[:], in_=w_up_h[:])

            # padded intermediates: c1/c2 single-group, c3 as kx-shifted X3
            c1pad = img.tile([32, NPAD], BF16, tag="c1pad")
            border_memsets(c1pad, nc.vector, nc.gpsimd, 2)
            c2pad = img.tile([32, NPAD], BF16, tag="c2pad")
            border_memsets(c2pad, nc.vector, nc.gpsimd, 2)
            x3c3 = img.tile([96, NPAD], BF16, tag="x3c3")
            border_memsets(x3c3, nc.vector, nc.gpsimd, 4)

            # ---- style load + stats --------------------------------------
            # per-chunk slots: 0 tt, 1 r0, 2 r63, 3 c0, 4 c63,
            #                  5 c0a(y>=1), 6 c0b(y<=62), 7 c63a, 8 c63b
            stats = const.tile([128, NCH * 16], F32, tag="stats")
            sdump = const.tile([128, NPIX], BF16, tag="sdump")
            for c in range(NCH):
                st = styl[c]
                syx = st[:].rearrange("p (y x) -> p y x", x=W)
                sl = stats[:, c * 16:c * 16 + 16]
                rd = nc.vector.tensor_reduce
                if c % 2 == 0:
                    rd(sl[:, 0:1], syx, mybir.AxisListType.XY, ADD)
                else:
                    # whole-image sum on the Act engine (accum_out)
                    nc.scalar.activation(
                        sdump[:], st[:], mybir.ActivationFunctionType.Copy,
                        accum_out=sl[:, 0:1],
                    )
                rd(sl[:, 1:2], syx[:, 0, :], mybir.AxisListType.X, ADD)
                rd(sl[:, 2:3], syx[:, H - 1, :], mybir.AxisListType.X, ADD)
                rd(sl[:, 3:4], syx[:, :, 0], mybir.AxisListType.X, ADD)
                rd(sl[:, 4:5], syx[:, :, W - 1], mybir.AxisListType.X, ADD)
                rd(sl[:, 5:6], syx[:, 1:H, 0], mybir.AxisListType.X, ADD)
                rd(sl[:, 6:7], syx[:, 0:H - 1, 0], mybir.AxisListType.X, ADD)
                rd(sl[:, 7:8], syx[:, 1:H, W - 1], mybir.AxisListType.X, ADD)
                rd(sl[:, 8:9], syx[:, 0:H - 1, W - 1], mybir.AxisListType.X, ADD)

            # ---- R assembly: R[:, c*9+t] --------------------------------
            R = const.tile([128, 36], F32, tag="R")
            Rb = const.tile([128, 36], BF16, tag="Rb")

            def st4(slot):
                return stats[:].rearrange("p (c s) -> p c s", s=16)[:, :, slot]

            def R4(t):
                return R[:].rearrange("p (c t) -> p c t", t=9)[:, :, t]

            v = nc.vector
            v.tensor_copy(R4(4), st4(0))                   # (1,1)
            v.tensor_sub(R4(1), st4(0), st4(2))            # (0,1): tt-r63
            v.tensor_sub(R4(7), st4(0), st4(1))            # (2,1): tt-r0
            v.tensor_sub(R4(3), st4(0), st4(4))            # (1,0): tt-c63
            v.tensor_sub(R4(5), st4(0), st4(3))            # (1,2): tt-c0
            v.tensor_sub(R4(0), R4(1), st4(8))             # (0,0): R1-c63b
            v.tensor_sub(R4(2), R4(1), st4(6))             # (0,2): R1-c0b
            v.tensor_sub(R4(6), R4(7), st4(7))             # (2,0): R7-c63a
            v.tensor_sub(R4(8), R4(7), st4(5))             # (2,2): R7-c0a
            v.tensor_copy(Rb[:], R[:])

            # ---- ds conv part 1 (tiles 0..4) -----------------------------
            def ds_tile(rt):
                pool = cps if rt % 2 == 0 else dps
                ps = pool.tile([128, NT], F32, tag=pool.name)
                psr = ps[:].rearrange("p (r x) -> p r x", x=W)
                for w in range(9):
                    for g in range(4):
                        jj = w * 4 + g
                        c, t = divmod(jj, 9)
                        ky, kx = divmod(t, 3)
                        rhs = _yx(cpad[c][:])[:, rt * TR + ky:rt * TR + ky + TR,
                                              kx:kx + W]
                        nc.tensor.matmul(
                            psr[32 * g:32 * g + 32],
                            w_ds_sb[:, jj * 32:(jj + 1) * 32],
                            rhs,
                            start=(w == 0), stop=(w == 8),
                            tile_position=(0, 32 * g),
                        )
                # drain: sum 4 col-group partials + bias -> X3 center
                # (PSUM readable only by DVE/Act, max one PSUM operand each)
                s1 = drain.tile([32, NT], F32, tag="s1")
                nc.scalar.activation(
                    s1[:], psr[32:64], mybir.ActivationFunctionType.Copy
                )
                s2 = drain.tile([32, NT], F32, tag="s2")
                nc.scalar.activation(
                    s2[:], psr[64:96], mybir.ActivationFunctionType.Copy
                )
                a1 = drain.tile([32, NT], F32, tag="a1")
                nc.vector.tensor_add(a1[:], psr[0:32], s1[:])
                b1 = drain.tile([32, NT], F32, tag="b1")
                nc.vector.scalar_tensor_tensor(
                    b1[:], psr[96:128], dsb_sb[:], s2[:], op0=ADD, op1=ADD
                )
                rows = slice(rt * TR + 1, rt * TR + 1 + TR)
                ctr = _yx(c1pad[:])[:, rows, 1:1 + W]
                nc.gpsimd.tensor_add(
                    ctr, a1[:].rearrange("p (r x) -> p r x", x=W),
                    b1[:].rearrange("p (r x) -> p r x", x=W),
                )

            def x3_replicate(dst, band):
                # duplicate center group into kx=0 / kx=2 shifted groups.
                # flat +-1 shifted copies are contiguous, and the center
                # group's zeroed pad columns land on the borders correctly.
                a, b = (1, 33) if band == 0 else (33, 65)
                lo, hi = a * PW, b * PW
                src = dst[32:64, lo:hi]
                nc.sync.dma_start(out=dst[0:32, lo + 1:hi + 1], in_=src)
                nc.sync.dma_start(out=dst[64:96, lo - 1:hi - 1], in_=src)

            for rt in range(5):
                ds_tile(rt)

            # ---- predictor: pooled + FC (interleaved with ds conv) -------
            pps = pps_pool.tile([64, 1], F32, tag="pooled")
            for jj in range(36):
                nc.tensor.matmul(
                    pps[:], cw_sb[:, jj * 64:(jj + 1) * 64], Rb[:, jj:jj + 1],
                    start=(jj == 0), stop=(jj == 35),
                )
            lf = const.tile([66, 2], FP8, tag="lf")
            nc.sync.dma_start(out=lf[:], in_=lfi_h[:])
            # pooled is scaled by 64 so fp8 holds it with full precision;
            # fw rows are host-scaled by 64; FC drain divides by 4096.
            nc.scalar.activation(
                lf[0:32, 0:1], pps[0:32], mybir.ActivationFunctionType.Identity,
                bias=cb_sb[0:32], scale=64.0 / NPIX,
            )
            nc.scalar.activation(
                lf[32:64, 1:2], pps[32:64], mybir.ActivationFunctionType.Identity,
                bias=cb_sb[32:64], scale=64.0 / NPIX,
            )
            fsb = const.tile([2, 9216], BF16, tag="fsb")
            for s in range(18):
                fps = fps_pool.tile([2, NT], F32, tag="fc")
                nc.tensor.matmul(
                    fps[:], lf[:], fw_sb[:, s * NT:(s + 1) * NT],
                    start=True, stop=True,
                )
                if s % 2 == 0:
                    nc.vector.tensor_scalar_mul(
                        fsb[:, s * NT:(s + 1) * NT], fps[:], 1.0 / NPIX
                    )
                else:
                    nc.scalar.activation(
                        fsb[:, s * NT:(s + 1) * NT], fps[:],
                        mybir.ActivationFunctionType.Identity, scale=1.0 / NPIX,
                    )
            nc.sync.dma_start(out=fsc_h[:], in_=fsb[:])
            filt = []
            for F in range(2):
                ft = const.tile([32, 288], BF16, tag=f"filt{F}")
                nc.sync.dma_start(
                    out=ft[:], in_=fsc_h[F].rearrange("(p q) -> p q", q=288)
                )
                filt.append(ft)

            # ---- ds conv part 2 ------------------------------------------
            for rt in range(5, NRT):
                ds_tile(rt)

            # keep HAM warm across the filter-reload wait (no-dep matmuls)
            warm2 = fps_pool.tile([32, NT], F32, tag="fc")
            for i in range(10):
                nc.tensor.matmul(
                    warm2[:], w_ds_sb[:, 0:32], w_ds_sb[:, 0:NT],
                    start=True, stop=True,
                )

            # ---- dyn convs: 9 taps, 4 row-tiles packed in PE col groups --
            def dyn_conv(src, f, write_out):
                for sw in range(2):
                    ps = dps.tile([128, NT], F32, tag="dps")
                    psr = ps[:].rearrange("p (r x) -> p r x", x=W)
                    for g in range(4):
                        rt = sw * 4 + g
                        for t in range(9):
                            ky, kx = divmod(t, 3)
                            rhs = _yx(src[:])[:, rt * TR + ky:rt * TR + ky + TR,
                                              kx:kx + W]
                            nc.tensor.matmul(
                                psr[32 * g:32 * g + 32],
                                f[:, t * 32:(t + 1) * 32], rhs,
                                start=(t == 0), stop=(t == 8),
                                tile_position=(0, 32 * g),
                            )
                    for g in range(4):
                        write_out(sw * 4 + g, psr[32 * g:32 * g + 32])

            def dyn1_out(rt, psr):
                rows = slice(rt * TR + 1, rt * TR + 1 + TR)
                nc.scalar.activation(
                    _yx(c2pad[:])[:, rows, 1:1 + W], psr,
                    mybir.ActivationFunctionType.Lrelu, alpha=0.2,
                )

            def dyn2_out(rt, psr):
                rows = slice(rt * TR + 1, rt * TR + 1 + TR)
                ctr = _yx(x3c3[32:64])[:, rows, 1:1 + W]
                if rt % 2 == 0:
                    nc.scalar.activation(
                        ctr, psr, mybir.ActivationFunctionType.Copy
                    )
                else:
                    nc.vector.tensor_copy(ctr, psr)
                if rt == 3:
                    x3_replicate(x3c3, 0)
                elif rt == 7:
                    x3_replicate(x3c3, 1)

            dyn_conv(c1pad, filt[0], dyn1_out)
            dyn_conv(c2pad, filt[1], dyn2_out)

            # residual content (bf16), split across both rings
            cont = []
            for c in range(NCH):
                ct = img.tile([128, NPIX], BF16, tag=f"cont{c}")
                cq = nc.scalar if c % 2 == 0 else nc.sync
                cq.dma_start(
                    out=ct[:], in_=contf_h[:].rearrange("(c p) q -> c p q", p=128)[c]
                )
                cont.append(ct)

            # keep HAM warm across the x3 replicate / cont wait
            warm3 = fps_pool.tile([32, NT], F32, tag="fc")
            for i in range(8):
                nc.tensor.matmul(
                    warm3[:], w_ds_sb[:, 0:32], w_ds_sb[:, 0:NT],
                    start=True, stop=True,
                )

            # ---- up conv + residual --------------------------------------
            for cc in range(NCH):
                outt = sx.tile([128, NPIX], BF16, tag="sx")
                for rt in range(NRT):
                    pool = cps if rt % 2 == 0 else dps
                    ps = pool.tile([128, NT], F32, tag=pool.name)
                    psr = ps[:].rearrange("p (r x) -> p r x", x=W)
                    # residual: content folded in via 64*I matmul (the conv
                    # path carries a x64 scale from w_ds; drain divides)
                    nc.tensor.matmul(
                        ps[:], id_sb[:], cont[cc][:, rt * NT:(rt + 1) * NT],
                        start=True, stop=False,
                    )
                    for ky in range(3):
                        rhs = _yx(x3c3[0:96])[:, rt * TR + ky:rt * TR + ky + TR,
                                              1:1 + W]
                        nc.tensor.matmul(
                            psr, w_up_sb[:, (ky * NCH + cc) * 128:
                                         (ky * NCH + cc + 1) * 128], rhs,
                            start=False, stop=(ky == 2),
                        )
                    oseg = outt[:, rt * NT:(rt + 1) * NT]
                    if cc % 2 == 0:
                        nc.vector.tensor_scalar(
                            oseg, ps[:], 1.0 / 64.0, upb_sb[:, cc:cc + 1],
                            op0=mybir.AluOpType.mult, op1=ADD,
                        )
                    else:
                        nc.scalar.activation(
                            oseg, ps[:], mybir.ActivationFunctionType.Identity,
                            bias=upb_sb[:, cc:cc + 1], scale=1.0 / 64.0,
                        )
                oq = nc.sync if cc < 2 else nc.scalar
                oq.dma_start(
                    out=out_h[:].rearrange("(c p) q -> c p q", p=128)[cc],
                    in_=outt[:],
                )

    nc.compile()
    return nc


_NC_CACHE = None


def _get_nc():
    global _NC_CACHE
    if _NC_CACHE is None:
        _NC_CACHE = _build_program()
    return _NC_CACHE


def _prep_weights(ds_w, up_w, f1_cw, f2_cw, f1_fw, f2_fw, f1_fb, f2_fb):
    # w_ds block jj = c*9 + t: [p, o] = 64 * ds_w[o, c*128+p, t]  (fp8)
    X = ds_w.transpose(1, 2, 3, 0).reshape(CIN, 9, INNER) * 64.0   # (i, t, o)
    w_ds = np.ascontiguousarray(
        X.reshape(NCH, 128, 9, INNER).transpose(1, 0, 2, 3).reshape(128, 36 * INNER)
    ).astype(NP_FP8)
    # cw block jj = c*9 + t: [p, F*32+o] = fF_cw[o, c*128+p, t]
    cws = []
    for cw in (f1_cw, f2_cw):
        Y = cw.transpose(1, 2, 3, 0).reshape(CIN, 9, INNER)
        cws.append(Y.reshape(NCH, 128, 9, INNER).transpose(1, 0, 2, 3))
    cwm = np.concatenate(cws, axis=3).reshape(128, 36 * 64)        # [p,(c,t),(F,o)]
    cwm = np.ascontiguousarray(cwm).astype(NP_BF16)
    # w_up [kx*32+i, (ky*4+cc)*128+oc] = up_w[cc*128+oc, i, ky, kx]
    B = up_w.reshape(NCH, 128, INNER, 3, 3).transpose(4, 2, 3, 0, 1)
    w_up = np.ascontiguousarray(B.reshape(96, 3 * NCH * 128)).astype(NP_BF16)
    # fw rows: 0-31 f1, 32-63 f2, 64 fb1, 65 fb2; col n = ((kx*32+i)*3+ky)*32+o
    def permw(fw):
        # fw [(o,i,ky,kx), k] -> [k, (i,ky,kx,o)]
        Z = fw.reshape(INNER, INNER, 3, 3, INNER).transpose(1, 2, 3, 0, 4)
        return Z.reshape(9216, INNER).T

    def permb(fb):
        return fb.reshape(INNER, INNER, 3, 3).transpose(1, 2, 3, 0).reshape(9216)

    # fw rows x64 / bias rows x4096 so fp8 storage keeps precision;
    # the on-chip FC drain divides by 4096 (lf carries pooled x64).
    fwm = np.zeros((66, 9216), np.float32)
    fwm[0:32] = permw(f1_fw) * 64.0
    fwm[32:64] = permw(f2_fw) * 64.0
    fwm[64] = permb(f1_fb) * 4096.0
    fwm[65] = permb(f2_fb) * 4096.0
    fwm = np.ascontiguousarray(fwm).astype(NP_FP8)
    return w_ds, cwm, w_up, fwm


def kernel(content, style, ds_w, ds_b, up_w, up_b,
           f1_cw, f1_cb, f1_fw, f1_fb,
           f2_cw, f2_cb, f2_fw, f2_fb):
    content = np.asarray(content, np.float32)
    style = np.asarray(style, np.float32)
    B = content.shape[0]
    assert B == 8

    w_ds, cwm, w_up, fwm = _prep_weights(
        np.asarray(ds_w, np.float32), np.asarray(up_w, np.float32),
        np.asarray(f1_cw, np.float32), np.asarray(f2_cw, np.float32),
        np.asarray(f1_fw, np.float32), np.asarray(f2_fw, np.float32),
        np.asarray(f1_fb, np.float32), np.asarray(f2_fb, np.float32))
    upb = np.ascontiguousarray(
        np.asarray(up_b, np.float32).reshape(NCH, 128).T)
    cb = np.concatenate([np.asarray(f1_cb, np.float32),
                         np.asarray(f2_cb, np.float32)]) * 64.0

    lfi = np.zeros((66, 2), np.float32)
    lfi[64, 0] = 1.0
    lfi[65, 1] = 1.0
    shared = {
        "w_ds": w_ds, "cw": cwm, "w_up": w_up, "fw": fwm,
        "ds_b": np.asarray(ds_b, np.float32) * 64.0,
        "up_b": upb, "cb": cb, "lfinit": lfi.astype(NP_FP8),
        "ident": (np.eye(128, dtype=np.float32) * 64.0).astype(NP_BF16),
    }
    cont_pad = np.zeros((B, CIN, H + 2, PW), NP_FP8)
    cont_pad[:, :, 1:65, 1:65] = content.reshape(B, CIN, H, W).astype(NP_FP8)
    cont_pad = cont_pad.reshape(B, CIN, NPAD)
    cont_bf = content.reshape(B, CIN, NPIX).astype(NP_BF16)
    styl_bf = style.reshape(B, CIN, NPIX).astype(NP_FP8)
    in_maps = []
    for b in range(B):
        m = dict(shared)
        m["content"] = np.ascontiguousarray(cont_pad[b])
        m["contf"] = np.ascontiguousarray(cont_bf[b])
        m["style"] = np.ascontiguousarray(styl_bf[b])
        in_maps.append(m)

    nc = _get_nc()
    trace = bool(int(os.environ.get("KF_TRACE", "0")))
    res = run_bass_kernel_spmd(nc, in_maps, core_ids=list(range(B)), trace=trace)
    if trace and getattr(res, "exec_time_ns", None) is not None:
        print(f"HW exec time: {res.exec_time_ns} ns")
        kernel.last_exec_ns = res.exec_time_ns
    kernel.last_results = res
    out = np.stack([res.results[b]["out"].reshape(CIN, H, W) for b in range(B)])
    return out.astype(np.float32)


if __name__ == "__main__":
    _get_nc()
    print("program built + compiled OK")
